# revision 28
# baseline (speedup 1.0000x reference)
"""Trainium2 Bass kernel for nn_AttentionModule (sparse_attention).

Strategy v2 (8 NeuronCores, no collectives):
  core c -> batch b = c // 2, head-half hh = c % 2 (8 of 16 heads, all
  T=1024 queries).  Splitting heads instead of queries halves the K/V
  projection work (it was duplicated across the query-split pair) at the
  cost of a host-side pair-sum of the two partial out-projections.

  Each core computes, for its (b, hh) attn-dim slice A' = 512 (4 units
  g of 128 = 2 heads each):
    qT  [A',T] = Wq'^T @ x (+bq', LARoPE)
    kT  [A',L] = Wk'^T @ ctx^T (+bk', LARoPE)
    v'  [L,8*65] = ctx @ Wv' (+bv'), with a ones-column per head
    attn[g,l,h2] [128,1024] = exp((k q^T)/32 + mask_bias)   (softmax
                numerator; logits are tiny so no max-subtraction)
    o_ps [65,512] = v'_h^T @ attn  (row 64 = denominator)
    (deferred normalization: denominators gathered into dn[16,512],
     one 1/x = exp(-log(x)) on ACT, PE ones-matmul broadcast, then
     osb = osb_u * bcast * x_mask)
    y [D,T] (f32, partial) = Wo'^T @ osb
  Host: out[b] = y_{b,0} + y_{b,1} + bo x xm.

All matmuls run in bf16 (fp32 PSUM accumulation).

Measured on trn2 (8 cores concurrent, NTFF profile): HW exec time
167.8us/core (staged baseline of this session: 247.3us), relative
error vs the fp32 reference 4.1e-3 (absmax/scale) -- bf16 noise.
"""

import contextlib
import math
import os
import sys

import numpy as np


def _ensure_paths():
    for p in ("/opt/trn_rl_repo", "/root/.axon_site/_ro/trn_rl_repo"):
        if os.path.isdir(p) and p not in sys.path:
            sys.path.insert(0, p)


try:
    import concourse.bass as bass  # noqa: F401
except ImportError:
    _ensure_paths()

import ml_dtypes
import concourse.bass as bass
import concourse.tile as tile
from concourse import bacc, bass2jax, mybir

# Problem shapes (hardcoded per the module definition).
B = 4
T = 1024
L = 1024
DM = 1024   # d_model
AD = 1024   # full attn_dim
ADH = 512   # per-core attn-dim slice (8 heads)
H = 16
HD = 64     # head dim
G = 4       # 128-wide units per core (2 heads each)
N_CORES = 8
SCALE = 1.0 / math.sqrt(AD)  # note: module scales by sqrt(attn_dim)
ROPE_GAMMA = 10.0
ROTARY_BASE = 10000.0
MASK_BIAS = -30000.0  # exp(x + MASK_BIAS) underflows to exactly 0.0 in fp32

MDT = mybir.dt.bfloat16
NP_MDT = ml_dtypes.bfloat16
FP8 = mybir.dt.float8e4
NP_FP8 = ml_dtypes.float8_e4m3
F32 = mybir.dt.float32
WSCL = 1.0  # (fp8 experiment failed correctness; bf16 needs no scale)

AL = mybir.AluOpType
AF = mybir.ActivationFunctionType


def build_program():
    nc = bacc.Bacc("TRN2", target_bir_lowering=False, debug=False)

    def din(name, shape, dt):
        return nc.dram_tensor(name, shape, dt, kind="ExternalInput").ap()

    # big tensors arrive host-pretiled as [128, n*width] (partition p holds
    # row n*128+p of the logical tensor in column block n) so each is ONE
    # contiguous-per-partition DMA -- Sync-engine descriptor dispatch costs
    # ~4.6ns/segment and serialized dispatch of many small DMAs was gating
    # the whole front of the kernel
    xs = din("xs", [128, 8 * T], MDT)      # x[b]  (d_model x T)
    ctxT = din("ctxT", [128, 8 * L], MDT)  # context[b].T
    wq = din("wq", [128, 8 * ADH], MDT)    # Wq[:, a-slice]
    wk = din("wk", [128, 8 * ADH], MDT)
    wv = din("wv", [128, 8 * ADH], MDT)
    wo = din("wo", [128, 4 * DM], MDT)     # Wo[a-slice, :]
    bqc = din("bqc", [128, G], F32)        # bq'[g*128+p] at [p, g]
    bkc = din("bkc", [128, G], F32)
    bvr = din("bvr", [1, ADH], MDT)
    onesr = din("onesr", [1, 128], MDT)
    cosq = din("cosq", [128, T], MDT)
    sin2q = din("sin2q", [128, T], MDT)
    cosk = din("cosk", [128, L], MDT)
    sin2k = din("sin2k", [128, L], MDT)
    cmb = din("cmb", [128, 8], F32)        # key-mask bias per (p, l_tile)
    xmf16 = din("xmf16", [16, 512], F32)   # query mask, row r=g*4+th*2+h2
    perm = din("perm", [128, 128], MDT)    # partition permutation p -> p^32
    # bcast map per (g,th): out[p] = rx[4g+2th + p//64]  (lhsT slice gt*128)
    sel16 = din("sel16", [16, 8 * 128], MDT)
    y = nc.dram_tensor("y", [DM, T], F32, kind="ExternalOutput").ap()

    with tile.TileContext(nc) as tc, contextlib.ExitStack() as ctx:
        sb = ctx.enter_context(tc.tile_pool(name="sb", bufs=1))
        ps = ctx.enter_context(tc.tile_pool(name="ps", bufs=2, space="PSUM"))

        # ---- DMA, in consumption order ---------------------------------
        C = {}

        def cload(nm, ap):
            t = sb.tile(list(ap.shape), ap.dtype, tag=nm, name=f"c_{nm}", bufs=1)
            nc.sync.dma_start(t[:], ap)
            C[nm] = t

        def load_one(nm, ap, n, width, dt):
            t = sb.tile([128, n * width], dt, tag=nm, bufs=1, name=nm)
            nc.sync.dma_start(t[:], ap)
            return [t[:, d * width:(d + 1) * width] for d in range(n)]

        # dispatched in consumption order; Q-gating tensors first
        wq_t = load_one("wq", wq, 8, ADH, MDT)
        xs_t = load_one("xs", xs, 8, T, MDT)
        cload("bqc", bqc)
        cload("cosq", cosq)
        cload("sin2q", sin2q)
        cload("perm", perm)
        ctx_t = load_one("ctxT", ctxT, 8, L, MDT)
        wk_t = load_one("wk", wk, 8, ADH, MDT)
        cload("bkc", bkc)
        cload("cosk", cosk)
        cload("sin2k", sin2k)
        cload("cmb", cmb)
        wv_t = load_one("wv", wv, 8, ADH, MDT)
        cload("bvr", bvr)
        cload("onesr", onesr)
        cload("xmf16", xmf16)
        cload("sel16", sel16)
        wo_t = load_one("wo", wo, 4, DM, MDT)

        # ---- persistent tiles ------------------------------------------
        qT_t = [sb.tile([128, T], MDT, tag="qT", bufs=4, name=f"qT{g}")
                for g in range(G)]
        kT_t = [sb.tile([128, L], MDT, tag="kT", bufs=4, name=f"kT{g}")
                for g in range(G)]
        vP_t = [None] * 8
        attn_t = [[[None] * 2 for _ in range(8)] for _ in range(G)]
        osb_u = [sb.tile([128, T], F32, tag="osbu", bufs=4, name=f"osbu{g}")
                 for g in range(G)]
        # denominators: DVE can only write partition bases {0,32,64,96}, so
        # stage each row on partition 0 and DMA-scatter onto 16 partitions
        # for one batched 1/x
        dn16 = sb.tile([16, 512], F32, tag="dn16", bufs=1, name="dn16")

        # ---- Q phase: qT[g] = rope(Wq'^T @ x + bq') --------------------
        # The perm-matmul of a unit is emitted with the NEXT unit's
        # projection so the PE never waits on the DVE stt results.
        pend_q = []

        def q_unit(g, th):
            tsl = slice(th * 512, (th + 1) * 512)
            q_ps = ps.tile([128, 512], F32, tag="pp", bufs=4, name=f"qps{g}_{th}")
            for d in range(8):
                nc.tensor.matmul(
                    q_ps[:], wq_t[d][:, g * 128:(g + 1) * 128], xs_t[d][:, tsl],
                    start=(d == 0), stop=(d == 7),
                )
            wsb = sb.tile([128, 512], MDT, tag="ropeW", bufs=3, name=f"qw{g}_{th}")
            nc.vector.scalar_tensor_tensor(
                wsb[:], q_ps[:], C["bqc"][:, g:g + 1], C["sin2q"][:, tsl],
                op0=AL.add, op1=AL.mult,
            )
            asb = sb.tile([128, 512], MDT, tag="ropeA", bufs=3, name=f"qa{g}_{th}")
            nc.vector.scalar_tensor_tensor(
                asb[:], q_ps[:], C["bqc"][:, g:g + 1], C["cosq"][:, tsl],
                op0=AL.add, op1=AL.mult,
            )
            pend_q.append((g, th, wsb, asb))

        def q_flush():
            g, th, wsb, asb = pend_q.pop(0)
            tsl = slice(th * 512, (th + 1) * 512)
            pw_ps = ps.tile([128, 512], F32, tag="pp", bufs=4, name=f"qpw{g}_{th}")
            nc.tensor.matmul(pw_ps[:], C["perm"][:], wsb[:], start=True, stop=True)
            nc.vector.tensor_add(qT_t[g][:, tsl], pw_ps[:], asb[:])

        # ---- V phase unit: v'[l] = (ctx @ Wv' + bv' | ones) ------------
        def v_unit(l):
            v_ps = ps.tile([128, 512], F32, tag="pp", bufs=4, name=f"vps{l}")
            for d in range(8):
                nc.tensor.matmul(
                    v_ps[:], ctx_t[d][:, l * 128:(l + 1) * 128], wv_t[d][:],
                    start=(d == 0), stop=False,
                )
            nc.tensor.matmul(
                v_ps[:], C["onesr"][0:1, 0:128], C["bvr"][0:1, :],
                start=False, stop=True,
            )
            vt = sb.tile([128, 520], MDT, tag="vP", bufs=8, name=f"vP{l}")
            out_ap = vt[:, :].rearrange("p (h e) -> p h e", e=65)[:, :, 0:64]
            in_ap = v_ps[:].rearrange("p (h d) -> p h d", d=64)
            nc.vector.tensor_copy(out_ap, in_ap)
            ones_ap = vt[:, :].rearrange("p (h e) -> p h e", e=65)[:, :, 64:65]
            nc.gpsimd.memset(ones_ap, 1.0)
            vP_t[l] = vt

        # ---- kproj(g) halves: kT[g] = rope(Wk'^T @ ctx^T + bk') --------
        kpend = {}

        def kproj_half(g, lh):
            if lh == 0:
                kpend[g] = []
            sl = slice(lh * 512, (lh + 1) * 512)
            k_ps = ps.tile([128, 512], F32, tag="pp", bufs=4, name=f"kps{g}_{lh}")
            for d in range(8):
                nc.tensor.matmul(
                    k_ps[:], wk_t[d][:, g * 128:(g + 1) * 128], ctx_t[d][:, sl],
                    start=(d == 0), stop=(d == 7),
                )
            wsb = sb.tile([128, 512], MDT, tag="ropeW", bufs=3, name=f"kw{g}_{lh}")
            nc.vector.scalar_tensor_tensor(
                wsb[:], k_ps[:], C["bkc"][:, g:g + 1], C["sin2k"][:, sl],
                op0=AL.add, op1=AL.mult,
            )
            asb = sb.tile([128, 512], MDT, tag="ropeA", bufs=3, name=f"ka{g}_{lh}")
            nc.vector.scalar_tensor_tensor(
                asb[:], k_ps[:], C["bkc"][:, g:g + 1], C["cosk"][:, sl],
                op0=AL.add, op1=AL.mult,
            )
            kpend[g].append((lh, wsb, asb))

        def kproj_flush(g):
            lh, wsb, asb = kpend[g].pop(0)
            sl = slice(lh * 512, (lh + 1) * 512)
            pw_ps = ps.tile([128, 512], F32, tag="pp", bufs=4, name=f"kpw{g}_{lh}")
            nc.tensor.matmul(pw_ps[:], C["perm"][:], wsb[:], start=True, stop=True)
            nc.vector.tensor_add(kT_t[g][:, sl], pw_ps[:], asb[:])

        # ---- QK + exp chunk: attn[g][l][h2] ----------------------------
        def qk_unit(g, l):
            qk0 = ps.tile([128, 1024], F32, tag="qk", bufs=2, name=f"qk{g}_{l}_0")
            qk1 = ps.tile([128, 1024], F32, tag="qk", bufs=2, name=f"qk{g}_{l}_1")
            lsl = slice(l * 128, (l + 1) * 128)
            for th in range(2):
                tsl = slice(th * 512, (th + 1) * 512)
                nc.tensor.matmul(
                    qk0[:, tsl], kT_t[g][0:64, lsl], qT_t[g][0:64, tsl],
                    start=True, stop=True,
                )
                nc.tensor.matmul(
                    qk1[:, tsl], kT_t[g][64:128, lsl], qT_t[g][64:128, tsl],
                    start=True, stop=True,
                )
            for h2, qk in ((0, qk0), (1, qk1)):
                at = sb.tile([128, 1024], MDT, tag="attn", bufs=24,
                             name=f"at{g}_{l}_{h2}")
                nc.scalar.activation(
                    at[:], qk[:], AF.Exp, bias=C["cmb"][:, l:l + 1], scale=SCALE,
                )
                attn_t[g][l][h2] = at

        # ---- PV chunk: o(g,h2,th) + unnormalized evacuation ------------
        def pv_mms(g, h2, th, o_ps, lrange):
            h = 2 * g + h2
            tsl = slice(th * 512, (th + 1) * 512)
            for l in lrange:
                nc.tensor.matmul(
                    o_ps[:], vP_t[l][:, h * 65:h * 65 + 65],
                    attn_t[g][l][h2][:, tsl],
                    start=(l == 0), stop=(l == 7),
                )

        def pv_evac(g, h2, th, o_ps):
            tsl = slice(th * 512, (th + 1) * 512)
            r = 4 * g + 2 * th + h2
            stg = sb.tile([1, 512], F32, tag="dnstage", bufs=2, name=f"dnst{r}")
            nc.vector.tensor_copy(stg[:], o_ps[64:65, :])
            nc.sync.dma_start(dn16[r:r + 1, :], stg[:])
            nc.vector.tensor_copy(
                osb_u[g][h2 * 64:(h2 + 1) * 64, tsl], o_ps[0:64, :])

        def pv_unit(g, h2, th):
            o_ps = ps.tile([65, 512], F32, tag="pp", bufs=4, name=f"o{g}_{h2}_{th}")
            pv_mms(g, h2, th, o_ps, range(8))
            pv_evac(g, h2, th, o_ps)

        # ---- schedule --------------------------------------------------
        def interleave(qks, others):
            # spread `others` proportionally between QK chunks so the PE
            # never waits out a full exp and neither list dumps its tail
            # back-to-back
            n, m = len(qks), len(others)
            if n == 0 or m == 0:
                for f in qks + others:
                    f()
                return
            rest = list(others)
            acc = 0
            for j in range(n):
                qks[j]()
                acc += m
                while acc >= n and rest:
                    rest.pop(0)()
                    acc -= n
            for f in rest:
                f()

        # prologue: Q(0), Q(1), kproj(0)
        q_unit(0, 0)
        q_unit(0, 1)
        q_flush()
        q_unit(1, 0)
        q_flush()
        kproj_half(0, 0)
        q_flush()
        kproj_flush(0)
        q_unit(1, 1)
        q_flush()
        v_unit(0)
        qk_unit(0, 0)
        qk_unit(0, 1)

        # steps
        for s in range(5):
            qks = ([lambda l=l, g=s: qk_unit(g, l) for l in range(2, 8)]
                   if s < 4 else [])
            if s < 3:
                qks += [lambda l=l, g=s + 1: qk_unit(g, l)
                        for l in range(0, 2)]
            others = []
            if s == 0:
                others.append(lambda: (kproj_half(0, 1), kproj_flush(0)))
                others.append(lambda: kproj_half(1, 0))
                others.append(lambda: (kproj_half(1, 1), kproj_flush(1)))
                others.append(lambda: kproj_flush(1))
                for l in range(1, 8):
                    others.append(lambda l=l: v_unit(l))
            elif s == 1:
                others.append(lambda: kproj_half(2, 0))
                others.append(lambda: (kproj_half(2, 1), kproj_flush(2)))
                others.append(lambda: kproj_flush(2))
                others.append(lambda: q_unit(2, 0))
                others.append(lambda: (q_flush(), q_unit(2, 1)))
                others.append(lambda: q_flush())
                for h2 in range(2):
                    for th in range(2):
                        others.append(lambda h2=h2, th=th: pv_unit(0, h2, th))
            elif s == 2:
                others.append(lambda: kproj_half(3, 0))
                others.append(lambda: (kproj_half(3, 1), kproj_flush(3)))
                others.append(lambda: kproj_flush(3))
                others.append(lambda: q_unit(3, 0))
                others.append(lambda: (q_flush(), q_unit(3, 1)))
                others.append(lambda: q_flush())
                for h2 in range(2):
                    for th in range(2):
                        others.append(lambda h2=h2, th=th: pv_unit(1, h2, th))
            elif s == 3:
                # PV(2) full chunks early; PV(3) chains pipelined into the
                # QK(3)/exp drain (its 4 o_ps plus the 2 qk tiles exactly
                # fill PSUM since no kproj/Q/V runs here)
                for h2 in range(2):
                    for th in range(2):
                        others.append(lambda h2=h2, th=th: pv_unit(2, h2, th))
                pv3 = {}

                def pv3_alloc():
                    for h2 in range(2):
                        for th in range(2):
                            pv3[(h2, th)] = ps.tile(
                                [65, 512], F32, tag="pp", bufs=4,
                                name=f"o3_{h2}_{th}")

                def pv3_part(lrange):
                    for h2 in range(2):
                        for th in range(2):
                            pv_mms(3, h2, th, pv3[(h2, th)], lrange)

                def pv3_finish():
                    # pre-warm the Ln table set while ACT drains the last
                    # exps -- keeps the 1.3us ACT_TABLE_LOAD out of the
                    # tail-critical chain
                    scr = sb.tile([1, 1], F32, tag="lnwarm", bufs=1,
                                  name="lnwarm")
                    nc.scalar.activation(scr[:], C["xmf16"][0:1, 0:1], AF.Ln)
                    for h2 in range(2):
                        for th in range(2):
                            pv_evac(3, h2, th, pv3[(h2, th)])

                others.append(lambda: (pv3_alloc(), pv3_part(range(0, 2))))
                others.append(lambda: pv3_part(range(2, 4)))
                others.append(lambda: pv3_part(range(4, 6)))
                others.append(lambda: (pv3_part(range(6, 8)), pv3_finish()))
            else:
                pass
            interleave(qks, others)
            if s == 3:
                break

        # ---- tail: normalization + out-projection ----------------------
        # 1/dn = exp(-log(dn)) on ACT (both fns share a table set; DVE's
        # iterative reciprocal would cost 4.3us on this shape)
        lnv = sb.tile([16, 512], F32, tag="lnv", bufs=1, name="lnv")
        nc.scalar.activation(lnv[:], dn16[:], AF.Ln)
        rcp = sb.tile([16, 512], F32, tag="rcp", bufs=1, name="rcp")
        nc.scalar.activation(rcp[:], lnv[:], AF.Exp, scale=-1.0)
        rx = sb.tile([16, 512], MDT, tag="rx", bufs=1, name="rx")
        nc.vector.tensor_mul(rx[:], rcp[:], C["xmf16"][:])

        osb_t = [sb.tile([128, T], MDT, tag="osb", bufs=4, name=f"osb{g}")
                 for g in range(G)]

        def norm_mul(g, th):
            tsl = slice(th * 512, (th + 1) * 512)
            gt = 2 * g + th
            bc_ps = ps.tile([128, 512], F32, tag="pp", bufs=4, name=f"bc{g}_{th}")
            nc.tensor.matmul(bc_ps[:], C["sel16"][:, gt * 128:(gt + 1) * 128],
                             rx[:], start=True, stop=True)
            nc.vector.tensor_mul(osb_t[g][:, tsl], osb_u[g][:, tsl], bc_ps[:])

        y_r = y.rearrange("(n p) t -> n p t", p=128)

        def oproj(d, th):
            tsl = slice(th * 512, (th + 1) * 512)
            o_ps = ps.tile([128, 512], F32, tag="pp", bufs=4, name=f"ops{d}_{th}")
            for a in range(4):
                nc.tensor.matmul(
                    o_ps[:], wo_t[a][:, d * 128:(d + 1) * 128],
                    osb_t[a][:, tsl], start=(a == 0), stop=(a == 3),
                )
            yt = sb.tile([128, 512], F32, tag="outsb", bufs=4, name=f"yt{d}_{th}")
            nc.scalar.copy(yt[:], o_ps[:])  # ACT; keeps DVE free
            nc.sync.dma_start(y_r[d][:, tsl], yt[:])

        for g in range(G):
            norm_mul(g, 0)
        for d in range(8):
            if d < 4:
                norm_mul(d, 1)
            oproj(d, 0)
        for d in range(8):
            oproj(d, 1)

    nc.compile()
    return nc


_PROGRAM = None


def _get_program():
    global _PROGRAM
    if _PROGRAM is None:
        _PROGRAM = build_program()
    return _PROGRAM


def _host_prep(x, context, x_mask, context_mask, Wq, bq, Wk, bk, Wv, bv, Wo, bo):
    """Build the 8 per-core input maps."""
    f32 = np.float32
    x = np.asarray(x, f32)
    context = np.asarray(context, f32)
    xm = np.asarray(x_mask).astype(f32)        # [B,1,T]
    cm = np.asarray(context_mask).astype(f32)  # [B,1,L]

    len_q = xm.sum(axis=(1, 2))  # [B]
    len_k = cm.sum(axis=(1, 2))

    inv_freq = 1.0 / (ROTARY_BASE ** (np.arange(0, HD, 2, dtype=f32) / HD))
    theta = (inv_freq * ROPE_GAMMA).astype(f32)  # [32]

    p = np.arange(128)
    pm32 = p % 32
    sgn_sin2 = np.where((p % 64) < 32, 1.0, -1.0).astype(f32)[:, None]

    perm = np.zeros((128, 128), f32)
    perm[p, p ^ 32] = 1.0  # lhsT: out[m] = sum_k perm[k, m] * in[k] = in[m^32]

    sel16 = np.zeros((16, 8 * 128), f32)
    for g in range(G):
        for th in range(2):
            gt = 2 * g + th
            sel16[4 * g + 2 * th + 0, gt * 128:gt * 128 + 64] = 1.0
            sel16[4 * g + 2 * th + 1, gt * 128 + 64:(gt + 1) * 128] = 1.0

    Wq = np.asarray(Wq, f32)
    Wk = np.asarray(Wk, f32)
    Wv = np.asarray(Wv, f32)
    Wo = np.asarray(Wo, f32)
    bq = np.asarray(bq, f32)
    bk = np.asarray(bk, f32)
    bv = np.asarray(bv, f32)

    in_maps = []
    for c in range(N_CORES):
        b, hh = c // 2, c % 2
        asl = slice(hh * ADH, (hh + 1) * ADH)

        pos_q = np.arange(T, dtype=f32) / len_q[b]
        fr_q = pos_q[None, :] * theta[pm32][:, None]       # [128, T]
        pos_k = np.arange(L, dtype=f32) / len_k[b]
        fr_k = pos_k[None, :] * theta[pm32][:, None]       # [128, L]

        m = {}
        def pretile(a, n):
            # [n*128, w] -> [128, n*w]: partition p, col block d = row d*128+p
            w = a.shape[1]
            return np.ascontiguousarray(
                a.reshape(n, 128, w).transpose(1, 0, 2).reshape(128, n * w))

        m["xs"] = pretile(x[b], 8).astype(NP_MDT)
        m["ctxT"] = pretile(np.ascontiguousarray(context[b].T), 8).astype(NP_MDT)
        m["wq"] = pretile(Wq[:, asl], 8).astype(NP_MDT)
        m["wk"] = pretile(Wk[:, asl], 8).astype(NP_MDT)
        m["wv"] = pretile(Wv[:, asl], 8).astype(NP_MDT)
        m["wo"] = pretile(Wo[asl, :], 4).astype(NP_MDT)
        m["bqc"] = bq[asl].reshape(G, 128).T.copy()
        m["bkc"] = bk[asl].reshape(G, 128).T.copy()
        m["bvr"] = bv[asl].reshape(1, ADH).astype(NP_MDT)
        m["onesr"] = np.ones((1, 128), NP_MDT)
        m["perm"] = perm.astype(NP_MDT)
        m["sel16"] = sel16.astype(NP_MDT)
        m["cosq"] = np.cos(fr_q).astype(NP_MDT)
        m["sin2q"] = (np.sin(fr_q) * sgn_sin2).astype(NP_MDT)
        m["cosk"] = np.cos(fr_k).astype(NP_MDT)
        m["sin2k"] = (np.sin(fr_k) * sgn_sin2).astype(NP_MDT)
        # 0.0 where the key is valid, MASK_BIAS where masked
        m["cmb"] = ((cm[b, 0] - 1.0) * (-MASK_BIAS)).reshape(8, 128).T.copy().astype(f32)
        xmf16 = np.empty((16, 512), f32)
        for g in range(G):
            for th in range(2):
                for h2 in range(2):
                    xmf16[4 * g + 2 * th + h2] = xm[b, 0, th * 512:(th + 1) * 512]
        m["xmf16"] = xmf16
        in_maps.append(m)
    return in_maps


_JIT_CACHE = {}


def _run_concurrent(nc, in_maps, n_cores=N_CORES):
    """Run the same bass program on n_cores devices concurrently, one
    single-device PJRT execute per core.

    (run_bass_kernel_spmd's multi-core path uses an 8-device shard_map
    SPMD executable, which hangs under this axon terminal; per-device
    dispatch of the identical program is functionally equivalent for a
    collective-free kernel and works.)
    """
    import jax

    bass2jax.install_neuronx_cc_hook()
    key = id(nc)
    if key not in _JIT_CACHE:
        partition_name = (
            nc.partition_id_tensor.name if nc.partition_id_tensor else None
        )
        in_names, out_names, out_avals, zero_outs = [], [], [], []
        for alloc in nc.m.functions[0].allocations:
            if not isinstance(alloc, mybir.MemoryLocationSet):
                continue
            name = alloc.memorylocations[0].name
            if alloc.kind == "ExternalInput":
                if name != partition_name:
                    in_names.append(name)
            elif alloc.kind == "ExternalOutput":
                shape = tuple(alloc.tensor_shape)
                dtype = mybir.dt.np(alloc.dtype)
                out_names.append(name)
                out_avals.append(jax.core.ShapedArray(shape, dtype))
                zero_outs.append(np.zeros(shape, dtype))
        n_params = len(in_names)
        in_names_full = list(in_names) + list(out_names)
        if partition_name is not None:
            in_names_full.append(partition_name)
        in_names_full = tuple(in_names_full)

        def _body(*args):
            operands = list(args)
            if partition_name is not None:
                operands.append(bass2jax.partition_id_tensor())
            outs = bass2jax._bass_exec_p.bind(
                *operands, out_avals=tuple(out_avals), in_names=in_names_full,
                out_names=tuple(out_names), lowering_input_output_aliases=(),
                sim_require_finite=True, sim_require_nnan=True, nc=nc)
            return tuple(outs)

        donate_idx = tuple(range(n_params, n_params + len(out_names)))
        jfn = jax.jit(_body, donate_argnums=donate_idx, keep_unused=True)
        _JIT_CACHE[key] = (jfn, in_names, out_names, zero_outs)

    jfn, in_names, out_names, zero_outs = _JIT_CACHE[key]
    devices = jax.devices()[:n_cores]
    futs = []
    for c, dev in enumerate(devices):
        args = [jax.device_put(np.asarray(in_maps[c][nm]), dev) for nm in in_names]
        args += [jax.device_put(z, dev) for z in zero_outs]
        futs.append(jfn(*args))
    return [
        {nm: np.asarray(futs[c][i]) for i, nm in enumerate(out_names)}
        for c in range(n_cores)
    ]


def precompile():
    """AOT-compile the NEFF (client-side) without touching the data plane."""
    import jax

    nc = _get_program()
    _run_concurrent(nc, [], n_cores=0)  # populate _JIT_CACHE only
    jfn, in_names, out_names, zero_outs = _JIT_CACHE[id(nc)]
    specs = []
    for alloc in nc.m.functions[0].allocations:
        if not isinstance(alloc, mybir.MemoryLocationSet):
            continue
        name = alloc.memorylocations[0].name
        if alloc.kind == "ExternalInput" and name in in_names:
            specs.append((name, jax.ShapeDtypeStruct(
                tuple(alloc.tensor_shape), mybir.dt.np(alloc.dtype))))
    by_name = dict(specs)
    args = [by_name[nm] for nm in in_names]
    args += [jax.ShapeDtypeStruct(z.shape, z.dtype) for z in zero_outs]
    compiled = jfn.lower(*args).compile()
    return compiled


def kernel(x, context, x_mask, context_mask, Wq, bq, Wk, bk, Wv, bv, Wo, bo):
    nc = _get_program()
    in_maps = _host_prep(x, context, x_mask, context_mask,
                         Wq, bq, Wk, bk, Wv, bv, Wo, bo)
    results = _run_concurrent(nc, in_maps, N_CORES)

    xm = np.asarray(x_mask).astype(np.float32)  # [B,1,T]
    bo = np.asarray(bo, np.float32)
    out = np.empty((B, DM, T), np.float32)
    for b in range(B):
        out[b] = results[2 * b]["y"] + results[2 * b + 1]["y"]
        out[b] += bo[:, None] * xm[b, 0][None, :]
    return out


# revision 29
# speedup vs baseline: 1.0368x; 1.0368x over previous
"""Trainium2 Bass kernel for nn_AttentionModule (sparse_attention).

Strategy v2 (8 NeuronCores, no collectives):
  core c -> batch b = c // 2, head-half hh = c % 2 (8 of 16 heads, all
  T=1024 queries).  Splitting heads instead of queries halves the K/V
  projection work (it was duplicated across the query-split pair) at the
  cost of a host-side pair-sum of the two partial out-projections.

  Each core computes, for its (b, hh) attn-dim slice A' = 512 (4 units
  g of 128 = 2 heads each):
    qT  [A',T] = Wq'^T @ x (+bq', LARoPE)
    kT  [A',L] = Wk'^T @ ctx^T (+bk', LARoPE)
    v'  [L,8*65] = ctx @ Wv' (+bv'), with a ones-column per head
    attn[g,l,h2] [128,1024] = exp((k q^T)/32 + mask_bias)   (softmax
                numerator; logits are tiny so no max-subtraction)
    o_ps [65,512] = v'_h^T @ attn  (row 64 = denominator)
    (deferred normalization: denominators gathered into dn[16,512],
     one 1/x = exp(-log(x)) on ACT, PE ones-matmul broadcast, then
     osb = osb_u * bcast * x_mask)
    y [D,T] (f32, partial) = Wo'^T @ osb
  Host: out[b] = y_{b,0} + y_{b,1} + bo x xm.

All matmuls run in bf16 (fp32 PSUM accumulation).

Measured on trn2 (8 cores concurrent, NTFF profile): HW exec time
167.8us/core (staged baseline of this session: 247.3us), relative
error vs the fp32 reference 4.1e-3 (absmax/scale) -- bf16 noise.
"""

import contextlib
import math
import os
import sys

import numpy as np


def _ensure_paths():
    for p in ("/opt/trn_rl_repo", "/root/.axon_site/_ro/trn_rl_repo"):
        if os.path.isdir(p) and p not in sys.path:
            sys.path.insert(0, p)


try:
    import concourse.bass as bass  # noqa: F401
except ImportError:
    _ensure_paths()

import ml_dtypes
import concourse.bass as bass
import concourse.tile as tile
from concourse import bacc, bass2jax, mybir

# Problem shapes (hardcoded per the module definition).
B = 4
T = 1024
L = 1024
DM = 1024   # d_model
AD = 1024   # full attn_dim
ADH = 512   # per-core attn-dim slice (8 heads)
H = 16
HD = 64     # head dim
G = 4       # 128-wide units per core (2 heads each)
N_CORES = 8
SCALE = 1.0 / math.sqrt(AD)  # note: module scales by sqrt(attn_dim)
ROPE_GAMMA = 10.0
ROTARY_BASE = 10000.0
MASK_BIAS = -30000.0  # exp(x + MASK_BIAS) underflows to exactly 0.0 in fp32

MDT = mybir.dt.bfloat16
NP_MDT = ml_dtypes.bfloat16
FP8 = mybir.dt.float8e4
NP_FP8 = ml_dtypes.float8_e4m3
F32 = mybir.dt.float32
WSCL = 1.0  # (fp8 experiment failed correctness; bf16 needs no scale)

AL = mybir.AluOpType
AF = mybir.ActivationFunctionType


def build_program():
    nc = bacc.Bacc("TRN2", target_bir_lowering=False, debug=False)

    def din(name, shape, dt):
        return nc.dram_tensor(name, shape, dt, kind="ExternalInput").ap()

    # big tensors arrive host-pretiled as [128, n*width] (partition p holds
    # row n*128+p of the logical tensor in column block n) so each is ONE
    # contiguous-per-partition DMA -- Sync-engine descriptor dispatch costs
    # ~4.6ns/segment and serialized dispatch of many small DMAs was gating
    # the whole front of the kernel
    xs = din("xs", [128, 8 * T], MDT)      # x[b]  (d_model x T)
    ctxT = din("ctxT", [128, 8 * L], MDT)  # context[b].T
    wq = din("wq", [128, 8 * ADH], MDT)    # Wq[:, a-slice]
    wk = din("wk", [128, 8 * ADH], MDT)
    wv = din("wv", [128, 8 * ADH], MDT)
    wo = din("wo", [128, 4 * DM], MDT)     # Wo[a-slice, :]
    bqc = din("bqc", [128, G], F32)        # bq'[g*128+p] at [p, g]
    bkc = din("bkc", [128, G], F32)
    bvr = din("bvr", [1, ADH], MDT)
    onesr = din("onesr", [1, 128], MDT)
    cosq = din("cosq", [128, T], MDT)
    sin2q = din("sin2q", [128, T], MDT)
    cosk = din("cosk", [128, L], MDT)
    sin2k = din("sin2k", [128, L], MDT)
    cmb = din("cmb", [128, 8], F32)        # key-mask bias per (p, l_tile)
    xmf16 = din("xmf16", [16, 512], F32)   # query mask, row r=g*4+th*2+h2
    perm = din("perm", [128, 128], MDT)    # partition permutation p -> p^32
    # bcast map per (g,th): out[p] = rx[4g+2th + p//64]  (lhsT slice gt*128)
    sel16 = din("sel16", [16, 8 * 128], MDT)
    y = nc.dram_tensor("y", [DM, T], F32, kind="ExternalOutput").ap()

    with tile.TileContext(nc) as tc, contextlib.ExitStack() as ctx:
        sb = ctx.enter_context(tc.tile_pool(name="sb", bufs=1))
        ps = ctx.enter_context(tc.tile_pool(name="ps", bufs=2, space="PSUM"))

        # ---- DMA, in consumption order ---------------------------------
        C = {}

        def cload(nm, ap):
            t = sb.tile(list(ap.shape), ap.dtype, tag=nm, name=f"c_{nm}", bufs=1)
            nc.sync.dma_start(t[:], ap)
            C[nm] = t

        def load_one(nm, ap, n, width, dt):
            t = sb.tile([128, n * width], dt, tag=nm, bufs=1, name=nm)
            nc.sync.dma_start(t[:], ap)
            return [t[:, d * width:(d + 1) * width] for d in range(n)]

        # dispatched in consumption order; Q-gating tensors first
        wq_t = load_one("wq", wq, 8, ADH, MDT)
        xs_t = load_one("xs", xs, 8, T, MDT)
        cload("bqc", bqc)
        cload("cosq", cosq)
        cload("sin2q", sin2q)
        cload("perm", perm)
        ctx_t = load_one("ctxT", ctxT, 8, L, MDT)
        wk_t = load_one("wk", wk, 8, ADH, MDT)
        cload("bkc", bkc)
        cload("cosk", cosk)
        cload("sin2k", sin2k)
        cload("cmb", cmb)
        wv_t = load_one("wv", wv, 8, ADH, MDT)
        cload("bvr", bvr)
        cload("onesr", onesr)
        cload("xmf16", xmf16)
        cload("sel16", sel16)
        wo_t = load_one("wo", wo, 4, DM, MDT)

        # ---- persistent tiles ------------------------------------------
        qT_t = [sb.tile([128, T], MDT, tag="qT", bufs=4, name=f"qT{g}")
                for g in range(G)]
        kT_t = [sb.tile([128, L], MDT, tag="kT", bufs=4, name=f"kT{g}")
                for g in range(G)]
        vP_t = [None] * 8
        attn_t = [[[None] * 2 for _ in range(8)] for _ in range(G)]
        osb_u = [sb.tile([128, T], F32, tag="osbu", bufs=4, name=f"osbu{g}")
                 for g in range(G)]
        # denominators: DVE can only write partition bases {0,32,64,96}, so
        # stage each row on partition 0 and DMA-scatter onto 16 partitions
        # for one batched 1/x
        dn16 = sb.tile([16, 512], F32, tag="dn16", bufs=1, name="dn16")

        # ---- Q phase: qT[g] = rope(Wq'^T @ x + bq') --------------------
        # The perm-matmul of a unit is emitted with the NEXT unit's
        # projection so the PE never waits on the DVE stt results.
        pend_q = []

        def q_unit(g, th):
            tsl = slice(th * 512, (th + 1) * 512)
            q_ps = ps.tile([128, 512], F32, tag="pp", bufs=4, name=f"qps{g}_{th}")
            for d in range(8):
                nc.tensor.matmul(
                    q_ps[:], wq_t[d][:, g * 128:(g + 1) * 128], xs_t[d][:, tsl],
                    start=(d == 0), stop=(d == 7),
                )
            wsb = sb.tile([128, 512], MDT, tag="ropeW", bufs=3, name=f"qw{g}_{th}")
            nc.vector.scalar_tensor_tensor(
                wsb[:], q_ps[:], C["bqc"][:, g:g + 1], C["sin2q"][:, tsl],
                op0=AL.add, op1=AL.mult,
            )
            asb = sb.tile([128, 512], MDT, tag="ropeA", bufs=3, name=f"qa{g}_{th}")
            nc.vector.scalar_tensor_tensor(
                asb[:], q_ps[:], C["bqc"][:, g:g + 1], C["cosq"][:, tsl],
                op0=AL.add, op1=AL.mult,
            )
            pend_q.append((g, th, wsb, asb))

        def q_flush():
            g, th, wsb, asb = pend_q.pop(0)
            tsl = slice(th * 512, (th + 1) * 512)
            pw_ps = ps.tile([128, 512], F32, tag="pp", bufs=4, name=f"qpw{g}_{th}")
            nc.tensor.matmul(pw_ps[:], C["perm"][:], wsb[:], start=True, stop=True)
            nc.vector.tensor_add(qT_t[g][:, tsl], pw_ps[:], asb[:])

        # ---- V phase unit: v'[l] = (ctx @ Wv' + bv' | ones) ------------
        def v_unit(l):
            v_ps = ps.tile([128, 512], F32, tag="pp", bufs=4, name=f"vps{l}")
            for d in range(8):
                nc.tensor.matmul(
                    v_ps[:], ctx_t[d][:, l * 128:(l + 1) * 128], wv_t[d][:],
                    start=(d == 0), stop=False,
                )
            nc.tensor.matmul(
                v_ps[:], C["onesr"][0:1, 0:128], C["bvr"][0:1, :],
                start=False, stop=True,
            )
            vt = sb.tile([128, 520], MDT, tag="vP", bufs=8, name=f"vP{l}")
            out_ap = vt[:, :].rearrange("p (h e) -> p h e", e=65)[:, :, 0:64]
            in_ap = v_ps[:].rearrange("p (h d) -> p h d", d=64)
            nc.vector.tensor_copy(out_ap, in_ap)
            ones_ap = vt[:, :].rearrange("p (h e) -> p h e", e=65)[:, :, 64:65]
            nc.gpsimd.memset(ones_ap, 1.0)
            vP_t[l] = vt

        # ---- kproj(g) halves: kT[g] = rope(Wk'^T @ ctx^T + bk') --------
        kpend = {}

        def kproj_half(g, lh):
            if lh == 0:
                kpend[g] = []
            sl = slice(lh * 512, (lh + 1) * 512)
            k_ps = ps.tile([128, 512], F32, tag="pp", bufs=4, name=f"kps{g}_{lh}")
            for d in range(8):
                nc.tensor.matmul(
                    k_ps[:], wk_t[d][:, g * 128:(g + 1) * 128], ctx_t[d][:, sl],
                    start=(d == 0), stop=(d == 7),
                )
            wsb = sb.tile([128, 512], MDT, tag="ropeW", bufs=3, name=f"kw{g}_{lh}")
            nc.vector.scalar_tensor_tensor(
                wsb[:], k_ps[:], C["bkc"][:, g:g + 1], C["sin2k"][:, sl],
                op0=AL.add, op1=AL.mult,
            )
            asb = sb.tile([128, 512], MDT, tag="ropeA", bufs=3, name=f"ka{g}_{lh}")
            nc.vector.scalar_tensor_tensor(
                asb[:], k_ps[:], C["bkc"][:, g:g + 1], C["cosk"][:, sl],
                op0=AL.add, op1=AL.mult,
            )
            kpend[g].append((lh, wsb, asb))

        def kproj_flush(g):
            lh, wsb, asb = kpend[g].pop(0)
            sl = slice(lh * 512, (lh + 1) * 512)
            pw_ps = ps.tile([128, 512], F32, tag="pp", bufs=4, name=f"kpw{g}_{lh}")
            nc.tensor.matmul(pw_ps[:], C["perm"][:], wsb[:], start=True, stop=True)
            nc.vector.tensor_add(kT_t[g][:, sl], pw_ps[:], asb[:])

        # ---- QK + exp chunk: attn[g][l][h2] ----------------------------
        def qk_unit(g, l):
            qk0 = ps.tile([128, 1024], F32, tag="qk", bufs=2, name=f"qk{g}_{l}_0")
            qk1 = ps.tile([128, 1024], F32, tag="qk", bufs=2, name=f"qk{g}_{l}_1")
            lsl = slice(l * 128, (l + 1) * 128)
            for th in range(2):
                tsl = slice(th * 512, (th + 1) * 512)
                nc.tensor.matmul(
                    qk0[:, tsl], kT_t[g][0:64, lsl], qT_t[g][0:64, tsl],
                    start=True, stop=True,
                )
                nc.tensor.matmul(
                    qk1[:, tsl], kT_t[g][64:128, lsl], qT_t[g][64:128, tsl],
                    start=True, stop=True,
                )
            for h2, qk in ((0, qk0), (1, qk1)):
                at = sb.tile([128, 1024], MDT, tag="attn", bufs=24,
                             name=f"at{g}_{l}_{h2}")
                nc.scalar.activation(
                    at[:], qk[:], AF.Exp, bias=C["cmb"][:, l:l + 1], scale=SCALE,
                )
                attn_t[g][l][h2] = at

        # ---- PV chunk: o(g,h2,th) + unnormalized evacuation ------------
        def pv_mms(g, h2, th, o_ps, lrange):
            h = 2 * g + h2
            tsl = slice(th * 512, (th + 1) * 512)
            for l in lrange:
                nc.tensor.matmul(
                    o_ps[:], vP_t[l][:, h * 65:h * 65 + 65],
                    attn_t[g][l][h2][:, tsl],
                    start=(l == 0), stop=(l == 7),
                )

        def pv_evac(g, h2, th, o_ps):
            tsl = slice(th * 512, (th + 1) * 512)
            r = 4 * g + 2 * th + h2
            stg = sb.tile([1, 512], F32, tag="dnstage", bufs=2, name=f"dnst{r}")
            nc.vector.tensor_copy(stg[:], o_ps[64:65, :])
            nc.sync.dma_start(dn16[r:r + 1, :], stg[:])
            nc.vector.tensor_copy(
                osb_u[g][h2 * 64:(h2 + 1) * 64, tsl], o_ps[0:64, :])

        def pv_unit(g, h2, th):
            o_ps = ps.tile([65, 512], F32, tag="pp", bufs=4, name=f"o{g}_{h2}_{th}")
            pv_mms(g, h2, th, o_ps, range(8))
            pv_evac(g, h2, th, o_ps)

        # ---- schedule --------------------------------------------------
        def interleave(qks, others):
            # spread `others` proportionally between QK chunks so the PE
            # never waits out a full exp and neither list dumps its tail
            # back-to-back
            n, m = len(qks), len(others)
            if n == 0 or m == 0:
                for f in qks + others:
                    f()
                return
            rest = list(others)
            acc = 0
            for j in range(n):
                qks[j]()
                acc += m
                while acc >= n and rest:
                    rest.pop(0)()
                    acc -= n
            for f in rest:
                f()

        # prologue: Q(0), Q(1), kproj(0)
        q_unit(0, 0)
        q_unit(0, 1)
        q_flush()
        q_unit(1, 0)
        q_flush()
        kproj_half(0, 0)
        q_flush()
        kproj_flush(0)
        q_unit(1, 1)
        q_flush()
        v_unit(0)

        # steps
        for s in range(5):
            qks = ([lambda l=l, g=s: qk_unit(g, l) for l in range(8)]
                   if s < 4 else [])
            others = []
            if s == 0:
                others.append(lambda: (kproj_half(0, 1), kproj_flush(0)))
                others.append(lambda: kproj_half(1, 0))
                others.append(lambda: (kproj_half(1, 1), kproj_flush(1)))
                others.append(lambda: kproj_flush(1))
                for l in range(1, 8):
                    others.append(lambda l=l: v_unit(l))
            elif s == 1:
                others.append(lambda: kproj_half(2, 0))
                others.append(lambda: (kproj_half(2, 1), kproj_flush(2)))
                others.append(lambda: kproj_flush(2))
                others.append(lambda: q_unit(2, 0))
                others.append(lambda: (q_flush(), q_unit(2, 1)))
                others.append(lambda: q_flush())
                for h2 in range(2):
                    for th in range(2):
                        others.append(lambda h2=h2, th=th: pv_unit(0, h2, th))
            elif s == 2:
                others.append(lambda: kproj_half(3, 0))
                others.append(lambda: (kproj_half(3, 1), kproj_flush(3)))
                others.append(lambda: kproj_flush(3))
                others.append(lambda: q_unit(3, 0))
                others.append(lambda: (q_flush(), q_unit(3, 1)))
                others.append(lambda: q_flush())
                for h2 in range(2):
                    for th in range(2):
                        others.append(lambda h2=h2, th=th: pv_unit(1, h2, th))
            elif s == 3:
                # PV(2) full chunks early; PV(3) chains pipelined into the
                # QK(3)/exp drain (its 4 o_ps plus the 2 qk tiles exactly
                # fill PSUM since no kproj/Q/V runs here)
                for h2 in range(2):
                    for th in range(2):
                        others.append(lambda h2=h2, th=th: pv_unit(2, h2, th))
                pv3 = {}

                def pv3_alloc():
                    for h2 in range(2):
                        for th in range(2):
                            pv3[(h2, th)] = ps.tile(
                                [65, 512], F32, tag="pp", bufs=4,
                                name=f"o3_{h2}_{th}")

                def pv3_part(lrange):
                    for h2 in range(2):
                        for th in range(2):
                            pv_mms(3, h2, th, pv3[(h2, th)], lrange)

                def pv3_finish():
                    # pre-warm the Ln table set while ACT drains the last
                    # exps -- keeps the 1.3us ACT_TABLE_LOAD out of the
                    # tail-critical chain
                    scr = sb.tile([1, 1], F32, tag="lnwarm", bufs=1,
                                  name="lnwarm")
                    nc.scalar.activation(scr[:], C["xmf16"][0:1, 0:1], AF.Ln)
                    for h2 in range(2):
                        for th in range(2):
                            pv_evac(3, h2, th, pv3[(h2, th)])

                others.append(lambda: (pv3_alloc(), pv3_part(range(0, 2))))
                others.append(lambda: pv3_part(range(2, 4)))
                others.append(lambda: pv3_part(range(4, 6)))
                others.append(lambda: (pv3_part(range(6, 8)), pv3_finish()))
            else:
                pass
            interleave(qks, others)
            if s == 3:
                break

        # ---- tail: normalization + out-projection ----------------------
        # 1/dn = exp(-log(dn)) on ACT (both fns share a table set; DVE's
        # iterative reciprocal would cost 4.3us on this shape)
        lnv = sb.tile([16, 512], F32, tag="lnv", bufs=1, name="lnv")
        nc.scalar.activation(lnv[:], dn16[:], AF.Ln)
        rcp = sb.tile([16, 512], F32, tag="rcp", bufs=1, name="rcp")
        nc.scalar.activation(rcp[:], lnv[:], AF.Exp, scale=-1.0)
        rx = sb.tile([16, 512], MDT, tag="rx", bufs=1, name="rx")
        nc.vector.tensor_mul(rx[:], rcp[:], C["xmf16"][:])

        osb_t = [sb.tile([128, T], MDT, tag="osb", bufs=4, name=f"osb{g}")
                 for g in range(G)]

        def norm_mul(g, th):
            tsl = slice(th * 512, (th + 1) * 512)
            gt = 2 * g + th
            bc_ps = ps.tile([128, 512], F32, tag="pp", bufs=4, name=f"bc{g}_{th}")
            nc.tensor.matmul(bc_ps[:], C["sel16"][:, gt * 128:(gt + 1) * 128],
                             rx[:], start=True, stop=True)
            nc.vector.tensor_mul(osb_t[g][:, tsl], osb_u[g][:, tsl], bc_ps[:])

        y_r = y.rearrange("(n p) t -> n p t", p=128)

        def oproj(d, th):
            tsl = slice(th * 512, (th + 1) * 512)
            o_ps = ps.tile([128, 512], F32, tag="pp", bufs=4, name=f"ops{d}_{th}")
            for a in range(4):
                nc.tensor.matmul(
                    o_ps[:], wo_t[a][:, d * 128:(d + 1) * 128],
                    osb_t[a][:, tsl], start=(a == 0), stop=(a == 3),
                )
            yt = sb.tile([128, 512], F32, tag="outsb", bufs=4, name=f"yt{d}_{th}")
            nc.scalar.copy(yt[:], o_ps[:])  # ACT; keeps DVE free
            nc.sync.dma_start(y_r[d][:, tsl], yt[:])

        for g in range(G):
            norm_mul(g, 0)
        for d in range(8):
            if d < 4:
                norm_mul(d, 1)
            oproj(d, 0)
        for d in range(8):
            oproj(d, 1)

    nc.compile()
    return nc


_PROGRAM = None


def _get_program():
    global _PROGRAM
    if _PROGRAM is None:
        _PROGRAM = build_program()
    return _PROGRAM


def _host_prep(x, context, x_mask, context_mask, Wq, bq, Wk, bk, Wv, bv, Wo, bo):
    """Build the 8 per-core input maps."""
    f32 = np.float32
    x = np.asarray(x, f32)
    context = np.asarray(context, f32)
    xm = np.asarray(x_mask).astype(f32)        # [B,1,T]
    cm = np.asarray(context_mask).astype(f32)  # [B,1,L]

    len_q = xm.sum(axis=(1, 2))  # [B]
    len_k = cm.sum(axis=(1, 2))

    inv_freq = 1.0 / (ROTARY_BASE ** (np.arange(0, HD, 2, dtype=f32) / HD))
    theta = (inv_freq * ROPE_GAMMA).astype(f32)  # [32]

    p = np.arange(128)
    pm32 = p % 32
    sgn_sin2 = np.where((p % 64) < 32, 1.0, -1.0).astype(f32)[:, None]

    perm = np.zeros((128, 128), f32)
    perm[p, p ^ 32] = 1.0  # lhsT: out[m] = sum_k perm[k, m] * in[k] = in[m^32]

    sel16 = np.zeros((16, 8 * 128), f32)
    for g in range(G):
        for th in range(2):
            gt = 2 * g + th
            sel16[4 * g + 2 * th + 0, gt * 128:gt * 128 + 64] = 1.0
            sel16[4 * g + 2 * th + 1, gt * 128 + 64:(gt + 1) * 128] = 1.0

    Wq = np.asarray(Wq, f32)
    Wk = np.asarray(Wk, f32)
    Wv = np.asarray(Wv, f32)
    Wo = np.asarray(Wo, f32)
    bq = np.asarray(bq, f32)
    bk = np.asarray(bk, f32)
    bv = np.asarray(bv, f32)

    in_maps = []
    for c in range(N_CORES):
        b, hh = c // 2, c % 2
        asl = slice(hh * ADH, (hh + 1) * ADH)

        pos_q = np.arange(T, dtype=f32) / len_q[b]
        fr_q = pos_q[None, :] * theta[pm32][:, None]       # [128, T]
        pos_k = np.arange(L, dtype=f32) / len_k[b]
        fr_k = pos_k[None, :] * theta[pm32][:, None]       # [128, L]

        m = {}
        def pretile(a, n):
            # [n*128, w] -> [128, n*w]: partition p, col block d = row d*128+p
            w = a.shape[1]
            return np.ascontiguousarray(
                a.reshape(n, 128, w).transpose(1, 0, 2).reshape(128, n * w))

        m["xs"] = pretile(x[b], 8).astype(NP_MDT)
        m["ctxT"] = pretile(np.ascontiguousarray(context[b].T), 8).astype(NP_MDT)
        m["wq"] = pretile(Wq[:, asl], 8).astype(NP_MDT)
        m["wk"] = pretile(Wk[:, asl], 8).astype(NP_MDT)
        m["wv"] = pretile(Wv[:, asl], 8).astype(NP_MDT)
        m["wo"] = pretile(Wo[asl, :], 4).astype(NP_MDT)
        m["bqc"] = bq[asl].reshape(G, 128).T.copy()
        m["bkc"] = bk[asl].reshape(G, 128).T.copy()
        m["bvr"] = bv[asl].reshape(1, ADH).astype(NP_MDT)
        m["onesr"] = np.ones((1, 128), NP_MDT)
        m["perm"] = perm.astype(NP_MDT)
        m["sel16"] = sel16.astype(NP_MDT)
        m["cosq"] = np.cos(fr_q).astype(NP_MDT)
        m["sin2q"] = (np.sin(fr_q) * sgn_sin2).astype(NP_MDT)
        m["cosk"] = np.cos(fr_k).astype(NP_MDT)
        m["sin2k"] = (np.sin(fr_k) * sgn_sin2).astype(NP_MDT)
        # 0.0 where the key is valid, MASK_BIAS where masked
        m["cmb"] = ((cm[b, 0] - 1.0) * (-MASK_BIAS)).reshape(8, 128).T.copy().astype(f32)
        xmf16 = np.empty((16, 512), f32)
        for g in range(G):
            for th in range(2):
                for h2 in range(2):
                    xmf16[4 * g + 2 * th + h2] = xm[b, 0, th * 512:(th + 1) * 512]
        m["xmf16"] = xmf16
        in_maps.append(m)
    return in_maps


_JIT_CACHE = {}


def _run_concurrent(nc, in_maps, n_cores=N_CORES):
    """Run the same bass program on n_cores devices concurrently, one
    single-device PJRT execute per core.

    (run_bass_kernel_spmd's multi-core path uses an 8-device shard_map
    SPMD executable, which hangs under this axon terminal; per-device
    dispatch of the identical program is functionally equivalent for a
    collective-free kernel and works.)
    """
    import jax

    bass2jax.install_neuronx_cc_hook()
    key = id(nc)
    if key not in _JIT_CACHE:
        partition_name = (
            nc.partition_id_tensor.name if nc.partition_id_tensor else None
        )
        in_names, out_names, out_avals, zero_outs = [], [], [], []
        for alloc in nc.m.functions[0].allocations:
            if not isinstance(alloc, mybir.MemoryLocationSet):
                continue
            name = alloc.memorylocations[0].name
            if alloc.kind == "ExternalInput":
                if name != partition_name:
                    in_names.append(name)
            elif alloc.kind == "ExternalOutput":
                shape = tuple(alloc.tensor_shape)
                dtype = mybir.dt.np(alloc.dtype)
                out_names.append(name)
                out_avals.append(jax.core.ShapedArray(shape, dtype))
                zero_outs.append(np.zeros(shape, dtype))
        n_params = len(in_names)
        in_names_full = list(in_names) + list(out_names)
        if partition_name is not None:
            in_names_full.append(partition_name)
        in_names_full = tuple(in_names_full)

        def _body(*args):
            operands = list(args)
            if partition_name is not None:
                operands.append(bass2jax.partition_id_tensor())
            outs = bass2jax._bass_exec_p.bind(
                *operands, out_avals=tuple(out_avals), in_names=in_names_full,
                out_names=tuple(out_names), lowering_input_output_aliases=(),
                sim_require_finite=True, sim_require_nnan=True, nc=nc)
            return tuple(outs)

        donate_idx = tuple(range(n_params, n_params + len(out_names)))
        jfn = jax.jit(_body, donate_argnums=donate_idx, keep_unused=True)
        _JIT_CACHE[key] = (jfn, in_names, out_names, zero_outs)

    jfn, in_names, out_names, zero_outs = _JIT_CACHE[key]
    devices = jax.devices()[:n_cores]
    futs = []
    for c, dev in enumerate(devices):
        args = [jax.device_put(np.asarray(in_maps[c][nm]), dev) for nm in in_names]
        args += [jax.device_put(z, dev) for z in zero_outs]
        futs.append(jfn(*args))
    return [
        {nm: np.asarray(futs[c][i]) for i, nm in enumerate(out_names)}
        for c in range(n_cores)
    ]


def precompile():
    """AOT-compile the NEFF (client-side) without touching the data plane."""
    import jax

    nc = _get_program()
    _run_concurrent(nc, [], n_cores=0)  # populate _JIT_CACHE only
    jfn, in_names, out_names, zero_outs = _JIT_CACHE[id(nc)]
    specs = []
    for alloc in nc.m.functions[0].allocations:
        if not isinstance(alloc, mybir.MemoryLocationSet):
            continue
        name = alloc.memorylocations[0].name
        if alloc.kind == "ExternalInput" and name in in_names:
            specs.append((name, jax.ShapeDtypeStruct(
                tuple(alloc.tensor_shape), mybir.dt.np(alloc.dtype))))
    by_name = dict(specs)
    args = [by_name[nm] for nm in in_names]
    args += [jax.ShapeDtypeStruct(z.shape, z.dtype) for z in zero_outs]
    compiled = jfn.lower(*args).compile()
    return compiled


def kernel(x, context, x_mask, context_mask, Wq, bq, Wk, bk, Wv, bv, Wo, bo):
    nc = _get_program()
    in_maps = _host_prep(x, context, x_mask, context_mask,
                         Wq, bq, Wk, bk, Wv, bv, Wo, bo)
    results = _run_concurrent(nc, in_maps, N_CORES)

    xm = np.asarray(x_mask).astype(np.float32)  # [B,1,T]
    bo = np.asarray(bo, np.float32)
    out = np.empty((B, DM, T), np.float32)
    for b in range(B):
        out[b] = results[2 * b]["y"] + results[2 * b + 1]["y"]
        out[b] += bo[:, None] * xm[b, 0][None, :]
    return out
